# revision 18
# baseline (speedup 1.0000x reference)
"""Trainium2 Bass kernel for nn_AuxiliaryClustering (segment_reduce), v2.

Data-parallel over the batch dim on 8 NeuronCores. Per core, per 2048-row
tile, everything is row-major [128 part, 16 r, 64 k] and there are NO PE
transposes or gathers:
  - q = -2*Z@C^T + (c2 + OFFSET) entirely on the PE: 8 block-diag matmuls
    with a host-interleaved Z^T layout (lhsT) + one rank-2 const matmul
    that adds the (split-bf16) c2+OFFSET row vector.
  - one-hot via DVE max-tree + is_equal (A in fp16 so bf16-tie multi-hot
    is rare; selection uses MAX over the masked q so multi-hot rows stay
    harmless).
  - dist^2 = max_k(onehot * q) - OFFSET + z2;  z2 from ACT square of the
    interleaved Z + 8 tiny PE matmuls against a parity-selector.
  - per-cluster counts/distsum via reversed papp: stationary [ones, dist]
    columns, one-hot as the moving operand, PSUM-accumulated over all
    tiles. colsum(A) via an all-ones stationary, also PSUM-accumulated.
The [K]-sized partials are gathered to the host, summed across cores, and
the five scalars assembled there (f64).
"""

import os
from contextlib import ExitStack

import ml_dtypes
import numpy as np

import concourse.bass as bass
import concourse.bacc as bacc
import concourse.tile as tile
from concourse import mybir
from concourse.bass_utils import run_bass_kernel_spmd

F32 = mybir.dt.float32
BF16 = mybir.dt.bfloat16
FP16 = mybir.dt.float16
AX = mybir.AxisListType
OP = mybir.AluOpType
ACTF = mybir.ActivationFunctionType

B, D, K = 1000000, 64, 64
NCORES = 8
P = 128
R = 32
TILE_ROWS = P * R                                   # 2048
SHARD = B // NCORES                                 # 125000
NTILES = (SHARD + TILE_ROWS - 1) // TILE_ROWS       # 62
PAD_SHARD = NTILES * TILE_ROWS                      # 126976
NPAD = PAD_SHARD - SHARD                            # 1976
NCH = R // 2                                        # r-pair chunks per tile

EPS = 1e-08
WEIGHT = 0.1
OFFSET = 1024.0

# engine split for the two max-trees: levels 0..5 (sizes 32,16,8,4,2,1 out)
# True -> run that level on GpSimd instead of DVE
MAXTREE_GP = [False, False, False, False, False, False]
SELTREE_GP = [False, False, False, False, False, False]


def _tree(nc, pool, src, width0, dtype, gp_levels, tag):
    """Pairwise max tree over the innermost 64-wide axis of [P, R, 64]."""
    cur = src
    w = width0  # 64
    lvl = 0
    while w > 1:
        h = w // 2
        out = pool.tile([P, R, h], dtype, tag=f"{tag}{lvl}")
        eng = nc.gpsimd if gp_levels[lvl] else nc.vector
        eng.tensor_tensor(
            out=out[:], in0=cur[:, :, 0:h], in1=cur[:, :, h:w], op=OP.max,
        )
        cur = out
        w = h
        lvl += 1
    return cur  # [P, R, 1]


def build_nc(ntiles: int = NTILES):
    nc = bacc.Bacc("TRN2", target_bir_lowering=False, debug=False)

    a_d = nc.dram_tensor("a", [ntiles, P, R * K], FP16, kind="ExternalInput").ap()
    zz_d = nc.dram_tensor("zz", [ntiles, P, R * D], BF16, kind="ExternalInput").ap()
    bdc_d = nc.dram_tensor("bdc", [P, P], BF16, kind="ExternalInput").ap()
    c2duo_d = nc.dram_tensor("c2duo", [2, 1024], BF16, kind="ExternalInput").ap()
    parsel_d = nc.dram_tensor("parsel", [P, 2], BF16, kind="ExternalInput").ap()
    colsel_d = nc.dram_tensor("colsel", [P, 3], FP16, kind="ExternalInput").ap()
    ones2r_d = nc.dram_tensor("ones2r", [2, P], BF16, kind="ExternalInput").ap()
    # separation inputs (f32)
    c_d = nc.dram_tensor("c", [K, D], F32, kind="ExternalInput").ap()
    ct_d = nc.dram_tensor("ct", [D, K], F32, kind="ExternalInput").ap()
    mask_d = nc.dram_tensor("mask", [K, K], F32, kind="ExternalInput").ap()

    papp_d = nc.dram_tensor("papp", [3, P], F32, kind="ExternalOutput").ap()
    colsum_d = nc.dram_tensor("colsum", [2, 512], F32, kind="ExternalOutput").ap()
    sep_d = nc.dram_tensor("sep", [K, 1], F32, kind="ExternalOutput").ap()

    with tile.TileContext(nc) as tc, ExitStack() as ctx:
        iop = ctx.enter_context(tc.tile_pool(name="io", bufs=6))
        wp = ctx.enter_context(tc.tile_pool(name="work", bufs=5))
        cp = ctx.enter_context(tc.tile_pool(name="const", bufs=1))
        ps_q = ctx.enter_context(tc.tile_pool(name="ps_q", bufs=5, space="PSUM"))
        ps_z2 = ctx.enter_context(tc.tile_pool(name="ps_z2", bufs=1, space="PSUM"))
        ps_pa = ctx.enter_context(tc.tile_pool(name="ps_pa", bufs=1, space="PSUM"))
        ps_cs = ctx.enter_context(tc.tile_pool(name="ps_cs", bufs=1, space="PSUM"))

        # ---- constants ----
        bdc_t = cp.tile([P, P], BF16)
        nc.sync.dma_start(out=bdc_t[:], in_=bdc_d[:])
        c2duo_t = cp.tile([2, 1024], BF16)
        nc.sync.dma_start(out=c2duo_t[:], in_=c2duo_d[:])
        parsel_t = cp.tile([P, 2], BF16)
        nc.sync.dma_start(out=parsel_t[:], in_=parsel_d[:])
        colsel_t = cp.tile([P, 3], FP16)
        nc.sync.dma_start(out=colsel_t[:], in_=colsel_d[:])
        ones2r_t = cp.tile([2, P], BF16)
        nc.sync.dma_start(out=ones2r_t[:], in_=ones2r_d[:])
        ones_t = cp.tile([P, 1], F32)
        nc.vector.memset(ones_t[:], 1.0)


        # ---- separation loss (one-time, tiny; same as baseline) ----
        sep_t = cp.tile([K, 1], F32)
        c_t = cp.tile([K, D], F32)
        nc.sync.dma_start(out=c_t[:], in_=c_d[:])
        ct_t = cp.tile([D, K], F32)
        nc.sync.dma_start(out=ct_t[:], in_=ct_d[:])
        mask_t = cp.tile([K, K], F32)
        nc.sync.dma_start(out=mask_t[:], in_=mask_d[:])

        csq_t = cp.tile([K, D], F32)
        nc.vector.tensor_tensor(out=csq_t[:], in0=c_t[:], in1=c_t[:], op=OP.mult)
        csqc_t = cp.tile([K, 1], F32)
        nc.vector.reduce_sum(csqc_t[:], csq_t[:], axis=AX.X)
        ctsq_t = cp.tile([D, K], F32)
        nc.vector.tensor_tensor(out=ctsq_t[:], in0=ct_t[:], in1=ct_t[:], op=OP.mult)

        g_ps = ps_z2.tile([K, K], F32, tag="z2")
        nc.tensor.matmul(g_ps[:], ct_t[:], ct_t[:], start=True, stop=True)
        row_ps = ps_cs.tile([1, K], F32, tag="colsum")
        nc.tensor.matmul(row_ps[:], ones_t[0:D, :], ctsq_t[:], start=True, stop=True)

        t1_t = cp.tile([K, K], F32)
        nc.scalar.activation(
            out=t1_t[:], in_=g_ps[:], func=ACTF.Identity,
            bias=csqc_t[:], scale=-2.0,
        )
        csqr_sb = cp.tile([1, K], F32)
        nc.scalar.copy(out=csqr_sb[:], in_=row_ps[:])
        csqr_b = cp.tile([K, K], F32)
        nc.gpsimd.partition_broadcast(csqr_b[:], csqr_sb[:])
        d2m_t = cp.tile([K, K], F32)
        nc.vector.tensor_tensor(out=d2m_t[:], in0=t1_t[:], in1=csqr_b[:], op=OP.add)
        nc.vector.tensor_scalar_max(out=d2m_t[:], in0=d2m_t[:], scalar1=0.0)
        dm_t = cp.tile([K, K], F32)
        nc.scalar.sqrt(dm_t[:], d2m_t[:])
        nc.vector.tensor_tensor(out=dm_t[:], in0=dm_t[:], in1=mask_t[:], op=OP.mult)
        nc.vector.reduce_sum(sep_t[:], dm_t[:], axis=AX.X)

        # ---- persistent accumulators ----
        papp_ps = ps_pa.tile([3, P], F32, tag="papp")
        colsum_ps = ps_cs.tile([2, 512], F32, tag="colsum")

        # ---- main loop ----
        for i in range(ntiles):
            a_t = iop.tile([P, R, K], FP16, tag="a")
            nc.sync.dma_start(out=a_t[:], in_=a_d[i])
            zz_t = iop.tile([P, R * D], BF16, tag="zz")
            nc.sync.dma_start(out=zz_t[:], in_=zz_d[i])

            # --- q = -2 Z C^T + (c2 + OFFSET), f32 PSUM, four 512-col quarters ---
            qh_t = wp.tile([P, R, K], FP16, tag="qh")
            for h in range(4):
                q_ps = ps_q.tile([P, 512], F32, tag="q")
                for j in range(4):
                    jj = h * 4 + j
                    nc.tensor.matmul(
                        q_ps[:, j * P:(j + 1) * P],
                        zz_t[:, jj * P:(jj + 1) * P],
                        bdc_t[:],
                        start=True, stop=False,
                    )
                nc.tensor.matmul(
                    q_ps[:], ones2r_t[:], c2duo_t[:, 0:512],
                    start=False, stop=True,
                )
                nc.scalar.copy(
                    out=qh_t[:, h * (R // 4):(h + 1) * (R // 4), :].rearrange(
                        "p r k -> p (r k)"),
                    in_=q_ps[:],
                )

            # --- one-hot: rowmax (single 1x reduce) + is_equal (fp16 2x) ---
            mfin = wp.tile([P, R, 1], FP16, tag="mfin")
            nc.vector.tensor_reduce(
                out=mfin[:], in_=a_t[:], axis=AX.X, op=OP.max,
            )
            max2 = wp.tile([P, R, 2], FP16, tag="max2")
            nc.vector.tensor_copy(
                out=max2[:], in_=mfin[:].broadcast_to([P, R, 2]),
            )
            oh_t = wp.tile([P, R, K], FP16, tag="oh")
            nc.vector.tensor_tensor(
                out=oh_t[:].rearrange("p r (a b) -> p r a b", b=2),
                in0=a_t[:].rearrange("p r (a b) -> p r a b", b=2),
                in1=max2[:, :, None, :].broadcast_to([P, R, K // 2, 2]),
                op=OP.is_equal,
            )

            # --- selection: prod = oh * qh ; selred = max_k prod ---
            prod_t = wp.tile([P, R, K], FP16, tag="prod")
            nc.vector.tensor_tensor(
                out=prod_t[:], in0=oh_t[:], in1=qh_t[:], op=OP.mult,
            )
            selred = wp.tile([P, R, 1], FP16, tag="selred")
            nc.vector.tensor_reduce(
                out=selred[:], in_=prod_t[:], axis=AX.X, op=OP.max,
            )

            # --- z2 via ACT square + 8 tiny PE matmuls ---
            zsq_t = wp.tile([P, R * D], BF16, tag="zsq")
            nc.gpsimd.tensor_tensor(
                out=zsq_t[:], in0=zz_t[:], in1=zz_t[:], op=OP.mult,
            )
            z2_ps = ps_z2.tile([P, R], F32, tag="z2")
            z2_ps = z2_ps[:]
            for j in range(NCH):
                nc.tensor.matmul(
                    z2_ps[:, j * 2:(j + 1) * 2],
                    zsq_t[:, j * P:(j + 1) * P],
                    parsel_t[:],
                    start=True, stop=True,
                )

            # --- dist = sqrt(selred - OFFSET + z2) -> do_t dist columns ---
            d2_t = wp.tile([P, R], F32, tag="d2")
            nc.vector.scalar_tensor_tensor(
                out=d2_t[:],
                in0=selred[:].rearrange("p r one -> p (r one)"),
                scalar=-OFFSET,
                in1=z2_ps,
                op0=OP.add,
                op1=OP.add,
            )
            do_t = wp.tile([P, NCH, 3], FP16, tag="do")
            nc.vector.memset(do_t[:, :, 0:1], 1.0)
            nc.scalar.sqrt(
                do_t[:, :, 1:3],
                d2_t[:].rearrange("p (c two) -> p c two", two=2),
            )

            # --- per-cluster counts+distsum (reversed papp) ---
            oh2d = oh_t[:].rearrange("p r k -> p (r k)")
            for j in range(NCH):
                nc.tensor.matmul(
                    papp_ps,
                    do_t[:, j, :],
                    oh2d[:, j * P:(j + 1) * P],
                    start=(i == 0 and j == 0),
                    stop=(i == ntiles - 1 and j == NCH - 1),
                )

            # --- colsum(A) ---
            a2d = a_t[:].rearrange("p r k -> p (r k)")
            nc.tensor.matmul(
                colsum_ps[:], colsel_t[:, 1:3], a2d[:, 512:1024],
                start=(i == 0), stop=False,
            )
            nc.tensor.matmul(
                colsum_ps[0:1, :], colsel_t[:, 0:1], a2d[:, 0:512],
                start=False, stop=False,
            )
            nc.tensor.matmul(
                colsum_ps[:], colsel_t[:, 1:3], a2d[:, 1536:2048],
                start=False, stop=False,
            )
            nc.tensor.matmul(
                colsum_ps[0:1, :], colsel_t[:, 0:1], a2d[:, 1024:1536],
                start=False, stop=(i == ntiles - 1),
            )

        # ---- evict + write outputs ----
        papp_sb = cp.tile([3, P], F32)
        nc.vector.tensor_copy(out=papp_sb[:], in_=papp_ps)
        nc.sync.dma_start(out=papp_d[:], in_=papp_sb[:])
        colsum_sb = cp.tile([2, 512], F32)
        nc.vector.tensor_copy(out=colsum_sb[:], in_=colsum_ps[:])
        nc.sync.dma_start(out=colsum_d[:], in_=colsum_sb[:])
        nc.sync.dma_start(out=sep_d[:], in_=sep_t[:])

    nc.finalize()
    return nc


_NC_CACHE = {}


def _get_nc():
    if "nc" not in _NC_CACHE:
        _NC_CACHE["nc"] = build_nc()
    return _NC_CACHE["nc"]


def _host_prep(a_s, z_s, consts):
    m = dict(consts)
    m["a"] = a_s
    m["zz"] = z_s
    return m


def kernel(latent_z, cluster_assignments, cluster_centers):
    z = np.asarray(latent_z, dtype=np.float32)
    a = np.ascontiguousarray(np.asarray(cluster_assignments, dtype=np.float32))
    c = np.ascontiguousarray(np.asarray(cluster_centers, dtype=np.float32))

    cbf = c.astype(ml_dtypes.bfloat16).astype(np.float32)
    c2 = (c.astype(np.float64) ** 2).sum(1).astype(np.float32)          # [K]
    t = c2 + np.float32(OFFSET)
    thi = t.astype(ml_dtypes.bfloat16).astype(np.float32)
    tlo = (t - thi).astype(ml_dtypes.bfloat16).astype(np.float32)

    # block-diag(-2 C^T, -2 C^T): bdc[(par,d), (par,k)] = -2 c[k, d]
    bdc = np.zeros((P, P), dtype=ml_dtypes.bfloat16)
    bdc[:D, :K] = (-2.0 * cbf).T
    bdc[D:, K:] = (-2.0 * cbf).T
    # c2duo rows: [thi tiled over r | tlo tiled over r] at free index (r, k)
    c2duo = np.zeros((2, 1024), dtype=ml_dtypes.bfloat16)
    c2duo[0] = np.tile(thi, 1024 // K)
    c2duo[1] = np.tile(tlo, 1024 // K)
    parsel = np.zeros((P, 2), dtype=ml_dtypes.bfloat16)
    parsel[:D, 0] = 1.0
    parsel[D:, 1] = 1.0
    colsel = np.zeros((P, 3), dtype=np.float16)
    colsel[:, 0] = 1.0
    colsel[:, 2] = 1.0
    ones2r = np.ones((2, P), dtype=ml_dtypes.bfloat16)

    consts = {
        "bdc": bdc, "c2duo": c2duo, "parsel": parsel, "colsel": colsel,
        "ones2r": ones2r, "c": c, "ct": np.ascontiguousarray(c.T),
        "mask": (1.0 - np.eye(K, dtype=np.float32)),
    }

    in_maps = []
    for core in range(NCORES):
        lo, hi = core * SHARD, (core + 1) * SHARD
        a_s = np.zeros((PAD_SHARD, K), dtype=np.float16)
        a_s[:SHARD] = a[lo:hi].astype(np.float16)
        a_s[SHARD:, 0] = 1.0
        a_tiles = np.ascontiguousarray(
            a_s.reshape(NTILES, P, R * K))

        z_s = np.zeros((PAD_SHARD, D), dtype=ml_dtypes.bfloat16)
        z_s[:SHARD] = z[lo:hi].astype(ml_dtypes.bfloat16)
        # zz[t, par*64+d, j*128+p] = z[t*2048 + p*16 + 2j + par, d]
        zt = z_s.reshape(NTILES, P, NCH, 2, D)          # t, p, j, par, d
        zz = np.ascontiguousarray(
            zt.transpose(0, 3, 4, 2, 1).reshape(NTILES, P, R * D))
        in_maps.append(_host_prep(a_tiles, zz, consts))

    nc = _get_nc()
    trace = bool(int(os.environ.get("KERNEL_PROFILE", "0")))
    res = run_bass_kernel_spmd(
        nc, in_maps, list(range(NCORES)), trace=trace, trace_cores=[0],
    )
    if trace:
        _NC_CACHE["exec_time_ns"] = res.exec_time_ns
        print(f"HW exec time: {res.exec_time_ns} ns")

    # ---- host-side all-reduce of partials + final scalar math ----
    counts = np.zeros(K, np.float64)
    distsum = np.zeros(K, np.float64)
    colsum = np.zeros(K, np.float64)
    for r in res.results:
        pa = r["papp"].astype(np.float64)               # [3, 128]
        counts += pa[0, :K] + pa[0, K:]
        distsum += pa[1, :K] + pa[2, K:]
        cs = r["colsum"].astype(np.float64).reshape(-1, K)
        colsum += cs.sum(axis=0)
    sep_rowsum = res.results[0]["sep"].astype(np.float64)[:, 0]

    # pad-row corrections (argmax 0, z = 0)
    pad_d2 = np.float64(np.float16(np.float32(thi[0]) + np.float32(tlo[0])))
    pad_dist = np.float64(
        np.float32(ml_dtypes.bfloat16(np.sqrt(pad_d2 - OFFSET))))
    counts[0] -= NCORES * NPAD
    distsum[0] -= NCORES * NPAD * pad_dist
    colsum[0] -= NCORES * NPAD

    probs = colsum / B
    balance = float(np.sum((1.0 / K) * (np.log(1.0 / K) - np.log(probs + EPS))))
    separation = float(-np.sum(sep_rowsum) / (K * (K - 1)))
    nonempty = counts > 0
    per_mean = distsum / np.maximum(counts, 1.0)
    n_nonempty = float(nonempty.sum())
    compact = float(np.sum(np.where(nonempty, per_mean, 0.0)) / max(n_nonempty, 1.0))
    aux = WEIGHT * balance + WEIGHT * separation + WEIGHT * compact
    cluster_balance = float(np.std(probs, ddof=1))

    return (
        np.float32(aux),
        np.float32(balance),
        np.float32(separation),
        np.float32(compact),
        np.float32(cluster_balance),
    )
